# revision 23
# baseline (speedup 1.0000x reference)
"""Trainium2 Bass kernel for AdaptiveDiffAttention (fp8 DoubleRow version).

Pure data parallel across 8 NeuronCores: each core processes B/8 = 2048
samples with a replicated copy of the weights. No collectives.

Math per sample b (seq len 2, heads 4, head dim 256):
  tokens  = x.reshape(2, 1024)
  lam     = sigmoid(relu(x @ L1) @ L2)
  Q,K,V   = tokens @ W*  (per token)
  softmax over 2 keys => a_q = sigmoid(SCALE * <Q_q, K_0 - K_1>)
  w0_q = relu(a1_q - lam*a2_q); w1_q = relu((1-a1_q) - lam*(1-a2_q))
  A_q  = w0_q * V_0 + w1_q * V_1   (per head)
  out_q = A_q @ WO + tokens_q

All large GEMMs run in fp8(e4m3) with MatmulPerfMode.DoubleRow (k=256
per pass, ~1.7x TensorE throughput vs bf16). Weights are pre-scaled by
64 on the host so their values sit in fp8's normal range; the descales
are folded into activation scale parameters that already existed
(sigmoid score scale, relu evict scale, final PSUM evict scale).

The residual (+tokens) is added on the PE itself: an identity-matmul
streams a host-prescaled bf16 copy of x (x*512) into the WO PSUM
accumulation group, so the final evict is a single scalar-engine copy
with scale 1/512 and no f32 x DMA is needed.

Host-side prep inside kernel(): shard x, quantize weights to fp8*64,
build fp8 transposed x and token-diff tiles, and bf16 x*512.
"""

import sys

for _p in ("/opt/trn_rl_repo", "/root/.axon_site/_ro/trn_rl_repo"):
    if _p not in sys.path:
        sys.path.append(_p)

import numpy as np
import ml_dtypes

import concourse.bass as bass
import concourse.mybir as mybir
import bass_rust
from concourse.tile import TileContext
from concourse.masks import make_identity

F32 = mybir.dt.float32
BF16 = mybir.dt.bfloat16
FP8 = mybir.dt.float8e4

DIM = 2048
SD = 1024
H = 4
HD = 256
LH = 256
SCALE = HD ** -0.5
N_CORES = 8
B_FULL = 16384
B_CORE = B_FULL // N_CORES  # 2048

SW = 64.0          # host weight pre-scale (all fp8 weights stored as W*64)
SV_EVICT = 8.0     # V evicted as V*8 (psum holds V*64; evict scale 1/8)
SO = SV_EVICT * SW  # WO psum holds out_attn*512 (+ x*512 residual)

AluOp = mybir.AluOpType
ActFn = mybir.ActivationFunctionType
DR = mybir.MatmulPerfMode.DoubleRow


def split_excess_waits(nc, max_waits=1):
    """Walrus codegen in this container rejects >1 sync wait on CTRL-class
    instructions. Move excess waits onto chained nops before the offender."""
    for f in nc.m.functions:
        for bb in f.blocks:
            new_insts = []
            for inst in bb.instructions:
                si = inst.sync_info
                if si is not None and si.on_wait and len(si.on_wait) > max_waits:
                    waits = list(si.on_wait)
                    extra, keep = waits[:-max_waits], waits[-max_waits:]
                    for ci in range(0, len(extra), max_waits):
                        chunk = extra[ci:ci + max_waits]
                        nop = mybir.InstNoOp(name=f"{inst.name}-wsplit{ci}")
                        nop.engine = inst.engine
                        nop.sync_info = bass_rust.SyncInfo(
                            on_wait=chunk, on_update=[])
                        nc.register_instruction(nop, overwrite=True)
                        new_insts.append(nop)
                    inst.sync_info = bass_rust.SyncInfo(
                        on_wait=keep, on_update=list(si.on_update or []))
                new_insts.append(inst)
            bb.instructions = new_insts


def build_kernel(n_samples=B_CORE, repeats=1):
    """Build the single-core Bass graph. n_samples must be a multiple of 128."""
    assert n_samples % 128 == 0
    n_mtiles = n_samples // 128

    nc = bass.Bass()

    # x transposed per m-tile: [mt, feat_in_tile(p), ftile, b] fp8
    xtp_d = nc.declare_dram_parameter(
        "xtp", [n_mtiles, 128, 16, 128], FP8, isOutput=False)
    # token-diff transposed per m-tile: [mt, p, ftile, b] fp8
    xtd_d = nc.declare_dram_parameter(
        "xtd", [n_mtiles, 128, 8, 128], FP8, isOutput=False)
    # x * 512 natural layout, bf16 (residual streamed into WO psum)
    xres_d = nc.declare_dram_parameter(
        "xres", [n_samples, DIM], BF16, isOutput=False)
    w_d = {}
    for name, pname in (("q1", "WQ1_w"), ("k1", "WK1_w"), ("q2", "WQ2_w"),
                        ("k2", "WK2_w"), ("v", "WV_w"), ("o", "WO_w")):
        w_d[name] = nc.declare_dram_parameter(pname, [SD, SD], FP8,
                                              isOutput=False)
    l1_d = nc.declare_dram_parameter("L1_w", [DIM, LH], FP8, isOutput=False)
    l2r_d = nc.declare_dram_parameter("L2r", [128, LH], F32, isOutput=False)
    out_d = nc.declare_dram_parameter("out", [n_samples, DIM], F32, isOutput=True)

    with TileContext(nc) as tc:
        with (
            tc.tile_pool(name="const", bufs=1) as const,
            tc.tile_pool(name="xt", bufs=2) as xt_p,
            tc.tile_pool(name="xtd", bufs=2) as xtd_p,
            tc.tile_pool(name="xres", bufs=2) as xres_p,
            tc.tile_pool(name="kdp", bufs=2) as kd_p,
            tc.tile_pool(name="vbuf", bufs=2) as v_p,
            tc.tile_pool(name="ptmp", bufs=2) as ptmp_p,
            tc.tile_pool(name="small", bufs=2) as small_p,
            tc.tile_pool(name="hbuf", bufs=2) as h_p,
            tc.tile_pool(name="dpool", bufs=36) as d_p,
            tc.tile_pool(name="at", bufs=2) as at_p,
            tc.tile_pool(name="obuf", bufs=2) as o_p,
            tc.tile_pool(name="psum_big", bufs=3, space="PSUM") as ps_big,
            tc.tile_pool(name="psum_lam", bufs=2, space="PSUM") as ps_lam_p,
        ):
            # ---------------- resident weights (fp8, already *64) -------------
            w_sb = {}
            for name in ("q1", "k1", "q2", "k2", "v", "o"):
                wt = const.tile([128, 8, SD], FP8, name=f"w_{name}")
                wr = w_d[name].rearrange("(ko p) n -> p ko n", p=128)
                # split so early-k matmuls can start before the full DMA lands
                nc.sync.dma_start(wt[:, :4, :], wr[:, :4, :])
                nc.sync.dma_start(wt[:, 4:, :], wr[:, 4:, :])
                w_sb[name] = wt
            l1_sb = const.tile([128, 16, LH], FP8, name="l1")
            nc.sync.dma_start(
                l1_sb[:], l1_d.rearrange("(ko p) n -> p ko n", p=128))
            l2_rep = const.tile([128, LH], F32, name="l2rep")
            nc.sync.dma_start(l2_rep[:], l2r_d[:])

            id_bf16 = const.tile([128, 128], BF16, name="id16")
            make_identity(nc, id_bf16[:])

            # round-robin psum->sbuf eviction (with optional scale) between
            # ACT and DVE: 3 ACT : 1 DVE keeps both under the PE roofline
            evict_ctr = [0]

            def evict(dst, src, scale=None):
                evict_ctr[0] += 1
                if evict_ctr[0] % 4 == 0:
                    if scale is None:
                        nc.vector.tensor_copy(dst, src)
                    else:
                        nc.vector.tensor_scalar_mul(dst, src, scale)
                else:
                    if scale is None:
                        nc.scalar.copy(dst, src)
                    else:
                        nc.scalar.mul(dst, src, scale)

            # ---------------- main loop over 128-sample tiles ----------------
            # Software-pipelined: phase A(mt) = DMAs + all projection GEMMs +
            # the DVE/ACT score->dmat chain; phase B(mt) = combine + WO + out.
            # Emitting B(mt-1) after A(mt) keeps the in-order PE queue busy on
            # tile mt's GEMMs while tile mt-1's score chain completes.

            def phase_a(mt):
                st = {}
                xt = xt_p.tile([128, 16, 128], FP8, tag="xt", name="xt")
                nc.sync.dma_start(xt[:], xtp_d[mt])
                xtd = xtd_p.tile([128, 8, 128], FP8, tag="xtd", name="xtd")
                nc.sync.dma_start(xtd[:], xtd_d[mt])
                xres = xres_p.tile([128, DIM], BF16, tag="xres", name="xres")
                nc.sync.dma_start(xres[:], xres_d[mt * 128:mt * 128 + 128, :])
                st["xres"] = xres

                # K-diffs: Kdiff_s = (T0 - T1) @ WKs, fp8 DoubleRow.
                # k1/k2 interleaved so both stream from one xtd weight load.
                kpsw = {k: ps_big.tile([128, SD], F32, tag="pbig", name="pbig")
                        for k in ("k1", "k2")}
                for fi in range(0, 8, 2):
                    for kname in ("k1", "k2"):
                        for n in range(2):
                            nsl = slice(n * 512, (n + 1) * 512)
                            nc.tensor.matmul(
                                kpsw[kname][:, nsl], xtd[:, fi:fi + 2, :],
                                w_sb[kname][:, fi:fi + 2, nsl],
                                start=(fi == 0), stop=(fi == 6),
                                perf_mode=DR)
                kdiff = {}
                for kname in ("k1", "k2"):
                    kd = kd_p.tile([128, SD], BF16, tag=f"kd_{kname}",
                                   name=f"kd_{kname}")
                    kdiff[kname] = kd
                    evict(kd[:], kpsw[kname][:])

                # Projections: q1/q2/v and the lam-MLP hidden GEMM interleaved
                # per token so all seven MM streams share each stationary xt
                # load.  lam accumulates across both tokens (full x row).
                ps_lam = ps_lam_p.tile([128, LH], F32, tag="plam", name="plam")
                proj = {}
                for tok in range(2):
                    psw = {}
                    for name in ("q1", "q2", "v"):
                        psw[name] = ps_big.tile([128, SD], F32, tag="pbig",
                                                name="pbig")
                    for fi in range(0, 8, 2):
                        lhsT = xt[:, tok * 8 + fi:tok * 8 + fi + 2, :]
                        for name in ("q1", "q2", "v"):
                            for n in range(2):
                                nsl = slice(n * 512, (n + 1) * 512)
                                nc.tensor.matmul(
                                    psw[name][:, nsl], lhsT,
                                    w_sb[name][:, fi:fi + 2, nsl],
                                    start=(fi == 0), stop=(fi == 6),
                                    perf_mode=DR)
                        nc.tensor.matmul(
                            ps_lam[:], lhsT,
                            l1_sb[:, tok * 8 + fi:tok * 8 + fi + 2, :],
                            start=(tok == 0 and fi == 0),
                            stop=(tok == 1 and fi == 6),
                            perf_mode=DR)
                    for name in ("q1", "q2", "v"):
                        pool = v_p if name == "v" else ptmp_p
                        sb = pool.tile([128, SD], BF16, tag=f"{name}_{tok}",
                                       name=f"{name}_{tok}")
                        proj[(name, tok)] = sb
                        evict(sb[:], psw[name][:],
                              scale=(1.0 / SV_EVICT) if name == "v" else None)
                st["proj"] = proj

                # lambda = sigmoid(relu(H)/64 . L2)
                h_sb = h_p.tile([128, LH], F32, tag="h", name="h")
                nc.scalar.activation(h_sb[:], ps_lam[:], ActFn.Relu,
                                     scale=1.0 / SW)
                nc.vector.tensor_tensor(h_sb[:], h_sb[:], l2_rep[:], AluOp.mult)
                logit = small_p.tile([128, 1], F32, tag="logit", name="logit")
                nc.vector.tensor_reduce(logit[:], h_sb[:],
                                        axis=mybir.AxisListType.X, op=AluOp.add)
                lam = small_p.tile([128, 1], F32, tag="lam", name="lam")
                nc.scalar.activation(lam[:], logit[:], ActFn.Sigmoid)
                u = small_p.tile([128, 1], F32, tag="u", name="u")
                nc.vector.tensor_scalar(u[:], lam[:], -1.0, 1.0,
                                        AluOp.mult, AluOp.add)

                # scores: r = <Q_q, Kdiff> per head; a = sigmoid(SCALE*r/4096)
                a = {}
                for si, sname in enumerate(("1", "2")):
                    for q in range(2):
                        p = ptmp_p.tile([128, SD], BF16, tag="p", name="p")
                        nc.vector.tensor_tensor(
                            p[:], proj[(f"q{sname}", q)][:],
                            kdiff[f"k{sname}"][:], AluOp.mult)
                        r = small_p.tile([128, H], F32, tag=f"r{si}{q}",
                                         name=f"r{si}{q}")
                        nc.vector.tensor_reduce(
                            r[:], p.rearrange("b (h d) -> b h d", h=H),
                            axis=mybir.AxisListType.X, op=AluOp.add)
                        aa = small_p.tile([128, H], F32, tag=f"a{si}{q}",
                                          name=f"a{si}{q}")
                        nc.scalar.activation(aa[:], r[:], ActFn.Sigmoid,
                                             scale=float(SCALE / (SW * SW)))
                        a[(si, q)] = aa

                # diff-attn weights -> diag matrices (bf16, on DVE).
                # Build order matches the combine's (ft, kv, q) consumption.
                wq = {}
                for q in range(2):
                    t = small_p.tile([128, H], F32, tag=f"t{q}", name=f"t{q}")
                    nc.vector.tensor_scalar_mul(t[:], a[(1, q)][:], lam[:])
                    w0q = small_p.tile([128, H], F32, tag=f"w0{q}",
                                       name=f"w0{q}")
                    nc.vector.tensor_tensor(w0q[:], a[(0, q)][:], t[:],
                                            AluOp.subtract)
                    nc.vector.tensor_scalar_max(w0q[:], w0q[:], 0.0)
                    w1q = small_p.tile([128, H], F32, tag=f"w1{q}",
                                       name=f"w1{q}")
                    nc.vector.tensor_tensor(w1q[:], t[:], a[(0, q)][:],
                                            AluOp.subtract)
                    nc.vector.tensor_scalar(w1q[:], w1q[:], u[:], 0.0,
                                            AluOp.add, AluOp.max)
                    wq[(q, 0)] = w0q
                    wq[(q, 1)] = w1q
                dmats = {}
                for h in range(H):
                    for kv in range(2):
                        for q in range(2):
                            d = d_p.tile([128, 128], BF16, tag="dmat",
                                         name="dmat")
                            nc.vector.tensor_scalar_mul(
                                d[:], id_bf16[:], wq[(q, kv)][:, h:h + 1])
                            dmats[(q, h, kv)] = d
                st["dmats"] = dmats
                return st

            def phase_c(mt, st):
                # A_q^T via diag matmuls (bf16):
                #   A_q^T[ft] = V_0[:,ft].T @ D0[q,h] + V_1[:,ft].T @ D1[q,h]
                # Loop order (ft, kv, q): both q streams share each stationary
                # V slice load.
                proj, dmats = st["proj"], st["dmats"]
                at = {}
                psq = {}
                for q in range(2):
                    at[q] = at_p.tile([128, 8, 128], FP8, tag=f"at{q}",
                                      name=f"at{q}")
                    psq[q] = ps_big.tile([128, SD], F32, tag="pbig",
                                         name="pbig")
                for ft in range(8):
                    h = ft // 2
                    fsl = slice(ft * 128, (ft + 1) * 128)
                    for kv in range(2):
                        for q in range(2):
                            nc.tensor.matmul(
                                psq[q][:, fsl], proj[("v", kv)][:, fsl],
                                dmats[(q, h, kv)][:],
                                start=(kv == 0), stop=(kv == 1),
                                skip_group_check=(ft % 4 != 0))
                for q in range(2):
                    nc.scalar.copy(at[q][:],
                                   psq[q][:].rearrange("b (f c) -> b f c",
                                                       c=128))
                st["at"] = at

            def phase_w(mt, st):
                # out_q*512 = x_q*512 + A_q*8 @ WO*64 ; evict with scale 1/512
                # (residual x*512 added into PSUM by DVE; PE is the hot engine)
                at, xres = st["at"], st["xres"]
                r0 = mt * 128
                for q in range(2):
                    o_sb = o_p.tile([128, SD], F32, tag=f"o{q}", name=f"o{q}")
                    pso = ps_big.tile([128, SD], F32, tag="pbig", name="pbig")
                    for fi in range(0, 8, 2):
                        for n in range(2):
                            nsl = slice(n * 512, (n + 1) * 512)
                            nc.tensor.matmul(pso[:, nsl],
                                             at[q][:, fi:fi + 2, :],
                                             w_sb["o"][:, fi:fi + 2, nsl],
                                             start=(fi == 0), stop=(fi == 6),
                                             perf_mode=DR)
                    nc.vector.tensor_tensor(
                        pso[:], pso[:], xres[:, q * SD:(q + 1) * SD],
                        AluOp.add)
                    osl = slice(q * SD, (q + 1) * SD)
                    nc.scalar.mul(o_sb[:], pso[:], 1.0 / SO)
                    nc.sync.dma_start(out_d[r0:r0 + 128, osl], o_sb[:])

            # Two-stage software pipeline: emitting combine+WO of tile mt-1
            # after the projections of tile mt keeps the in-order PE queue
            # busy while tile mt-1's score chain completes on DVE/ACT.
            # (A deeper 3-stage split measured slightly worse on HW.)
            n_iters = n_mtiles * repeats
            prev = None
            for mt_rep in range(n_iters):
                mt = mt_rep % n_mtiles
                st = phase_a(mt)
                if prev is not None:
                    phase_c(prev[0], prev[1])
                    phase_w(prev[0], prev[1])
                prev = (mt, st)
            phase_c(prev[0], prev[1])
            phase_w(prev[0], prev[1])

    split_excess_waits(nc)
    return nc


_NC_CACHE = {}


def _get_nc(n_samples):
    if n_samples not in _NC_CACHE:
        _NC_CACHE[n_samples] = build_kernel(n_samples)
    return _NC_CACHE[n_samples]


def host_prep(inputs, n_samples=B_CORE):
    """Host-side shard + dtype/layout prep. Returns in_maps for 8 cores."""
    x = np.ascontiguousarray(np.asarray(inputs["x"], dtype=np.float32))
    assert x.shape[0] == N_CORES * n_samples and x.shape[1] == DIM
    f8 = ml_dtypes.float8_e4m3
    bf = ml_dtypes.bfloat16
    ws = {}
    for k in ("WQ1_w", "WK1_w", "WQ2_w", "WK2_w", "WV_w", "WO_w", "L1_w"):
        ws[k] = np.ascontiguousarray(
            (np.asarray(inputs[k], dtype=np.float32) * SW).astype(f8))
    l2rep = np.ascontiguousarray(
        np.broadcast_to(np.asarray(inputs["L2_w"], dtype=np.float32)
                        .reshape(1, LH), (128, LH)))
    n_mtiles = n_samples // 128
    # fp8 transposed x: [core, mt, b, ft, p] -> [core, mt, p, ft, b]
    x8 = x.astype(f8)
    xb = x8.reshape(N_CORES, n_mtiles, 128, 16, 128)
    xtp_all = np.ascontiguousarray(xb.transpose(0, 1, 4, 3, 2))
    # fp8 transposed token diff: diff in f32, then quantize
    xd = (x[:, :SD] - x[:, SD:]).astype(f8)
    xdb = xd.reshape(N_CORES, n_mtiles, 128, 8, 128)
    xtd_all = np.ascontiguousarray(xdb.transpose(0, 1, 4, 3, 2))
    # bf16 x*512 natural
    xres = (x * SO).astype(bf).reshape(N_CORES, n_samples, DIM)
    in_maps = []
    for c in range(N_CORES):
        m = {"xtp": xtp_all[c], "xtd": xtd_all[c],
             "xres": np.ascontiguousarray(xres[c]), "L2r": l2rep}
        m.update(ws)
        in_maps.append(m)
    return in_maps


def kernel(**inputs):
    from concourse.bass_utils import run_bass_kernel_spmd

    nc = _get_nc(B_CORE)
    in_maps = host_prep(inputs, B_CORE)
    res = run_bass_kernel_spmd(nc, in_maps, core_ids=list(range(N_CORES)))
    return np.concatenate([res.results[c]["out"] for c in range(N_CORES)], axis=0)


# revision 25
# speedup vs baseline: 1.1572x; 1.1572x over previous
"""Trainium2 Bass kernel for AdaptiveDiffAttention (fp8 DoubleRow version).

Pure data parallel across 8 NeuronCores: each core processes B/8 = 2048
samples with a replicated copy of the weights. No collectives.

Math per sample b (seq len 2, heads 4, head dim 256):
  tokens  = x.reshape(2, 1024)
  lam     = sigmoid(relu(x @ L1) @ L2)
  Q,K,V   = tokens @ W*  (per token)
  softmax over 2 keys => a_q = sigmoid(SCALE * <Q_q, K_0 - K_1>)
  w0_q = relu(a1_q - lam*a2_q); w1_q = relu((1-a1_q) - lam*(1-a2_q))
  A_q  = w0_q * V_0 + w1_q * V_1   (per head)
  out_q = A_q @ WO + tokens_q

All large GEMMs run in fp8(e4m3) with MatmulPerfMode.DoubleRow (k=256
per pass, ~1.7x TensorE throughput vs bf16). Weights are pre-scaled by
64 on the host so their values sit in fp8's normal range; the descales
are folded into activation scale parameters that already existed
(sigmoid score scale, relu evict scale, final PSUM evict scale).

The residual (+tokens) is added on the PE itself: an identity-matmul
streams a host-prescaled bf16 copy of x (x*512) into the WO PSUM
accumulation group, so the final evict is a single scalar-engine copy
with scale 1/512 and no f32 x DMA is needed.

Host-side prep inside kernel(): shard x, quantize weights to fp8*64,
build fp8 transposed x and token-diff tiles, and bf16 x*512.
"""

import sys

for _p in ("/opt/trn_rl_repo", "/root/.axon_site/_ro/trn_rl_repo"):
    if _p not in sys.path:
        sys.path.append(_p)

import numpy as np
import ml_dtypes

import concourse.bass as bass
import concourse.mybir as mybir
import bass_rust
from concourse.tile import TileContext
from concourse.masks import make_identity

F32 = mybir.dt.float32
BF16 = mybir.dt.bfloat16
FP8 = mybir.dt.float8e4

DIM = 2048
SD = 1024
H = 4
HD = 256
LH = 256
SCALE = HD ** -0.5
N_CORES = 8
B_FULL = 16384
B_CORE = B_FULL // N_CORES  # 2048

SW = 64.0          # host weight pre-scale (all fp8 weights stored as W*64)
SV_EVICT = 8.0     # V evicted as V*8 (psum holds V*64; evict scale 1/8)
SO = SV_EVICT * SW  # WO psum holds out_attn*512 (+ x*512 residual)

AluOp = mybir.AluOpType
ActFn = mybir.ActivationFunctionType
DR = mybir.MatmulPerfMode.DoubleRow


def split_excess_waits(nc, max_waits=1):
    """Walrus codegen in this container rejects >1 sync wait on CTRL-class
    instructions. Move excess waits onto chained nops before the offender."""
    for f in nc.m.functions:
        for bb in f.blocks:
            new_insts = []
            for inst in bb.instructions:
                si = inst.sync_info
                if si is not None and si.on_wait and len(si.on_wait) > max_waits:
                    waits = list(si.on_wait)
                    extra, keep = waits[:-max_waits], waits[-max_waits:]
                    for ci in range(0, len(extra), max_waits):
                        chunk = extra[ci:ci + max_waits]
                        nop = mybir.InstNoOp(name=f"{inst.name}-wsplit{ci}")
                        nop.engine = inst.engine
                        nop.sync_info = bass_rust.SyncInfo(
                            on_wait=chunk, on_update=[])
                        nc.register_instruction(nop, overwrite=True)
                        new_insts.append(nop)
                    inst.sync_info = bass_rust.SyncInfo(
                        on_wait=keep, on_update=list(si.on_update or []))
                new_insts.append(inst)
            bb.instructions = new_insts


def build_kernel(n_samples=B_CORE, repeats=1):
    """Build the single-core Bass graph. n_samples must be a multiple of 128."""
    assert n_samples % 128 == 0
    n_mtiles = n_samples // 128

    nc = bass.Bass()

    # x transposed per m-tile: [mt, feat_in_tile(p), ftile, b] fp8
    xtp_d = nc.declare_dram_parameter(
        "xtp", [n_mtiles, 128, 16, 128], FP8, isOutput=False)
    # token-diff transposed per m-tile: [mt, p, ftile, b] fp8
    xtd_d = nc.declare_dram_parameter(
        "xtd", [n_mtiles, 128, 8, 128], FP8, isOutput=False)
    # x * 512 natural layout, bf16 (residual streamed into WO psum)
    xres_d = nc.declare_dram_parameter(
        "xres", [n_samples, DIM], BF16, isOutput=False)
    w_d = {}
    for name, pname in (("q1", "WQ1_w"), ("k1", "WK1_w"), ("q2", "WQ2_w"),
                        ("k2", "WK2_w"), ("v", "WV_w"), ("o", "WO_w")):
        w_d[name] = nc.declare_dram_parameter(pname, [SD, SD], FP8,
                                              isOutput=False)
    l1_d = nc.declare_dram_parameter("L1_w", [DIM, LH], FP8, isOutput=False)
    l2r_d = nc.declare_dram_parameter("L2r", [128, LH], F32, isOutput=False)
    out_d = nc.declare_dram_parameter("out", [n_samples, DIM], F32, isOutput=True)

    with TileContext(nc) as tc:
        with (
            tc.tile_pool(name="const", bufs=1) as const,
            tc.tile_pool(name="xt", bufs=2) as xt_p,
            tc.tile_pool(name="xtd", bufs=2) as xtd_p,
            tc.tile_pool(name="xres", bufs=2) as xres_p,
            tc.tile_pool(name="kdp", bufs=2) as kd_p,
            tc.tile_pool(name="vbuf", bufs=2) as v_p,
            tc.tile_pool(name="ptmp", bufs=2) as ptmp_p,
            tc.tile_pool(name="small", bufs=2) as small_p,
            tc.tile_pool(name="hbuf", bufs=2) as h_p,
            tc.tile_pool(name="dpool", bufs=36) as d_p,
            tc.tile_pool(name="at", bufs=2) as at_p,
            tc.tile_pool(name="obuf", bufs=2) as o_p,
            tc.tile_pool(name="psum_big", bufs=3, space="PSUM") as ps_big,
            tc.tile_pool(name="psum_lam", bufs=2, space="PSUM") as ps_lam_p,
        ):
            # ---------------- resident weights (fp8, already *64) -------------
            w_sb = {}
            for name in ("q1", "k1", "q2", "k2", "v", "o"):
                wt = const.tile([128, 8, SD], FP8, name=f"w_{name}")
                wr = w_d[name].rearrange("(ko p) n -> p ko n", p=128)
                # split so early-k matmuls can start before the full DMA lands
                nc.sync.dma_start(wt[:, :4, :], wr[:, :4, :])
                nc.sync.dma_start(wt[:, 4:, :], wr[:, 4:, :])
                w_sb[name] = wt
            l1_sb = const.tile([128, 16, LH], FP8, name="l1")
            nc.sync.dma_start(
                l1_sb[:], l1_d.rearrange("(ko p) n -> p ko n", p=128))
            l2_rep = const.tile([128, LH], F32, name="l2rep")
            nc.sync.dma_start(l2_rep[:], l2r_d[:])

            id_bf16 = const.tile([128, 128], BF16, name="id16")
            make_identity(nc, id_bf16[:])

            # round-robin psum->sbuf eviction (with optional scale) between
            # ACT and DVE: 3 ACT : 1 DVE keeps both under the PE roofline
            evict_ctr = [0]

            def evict(dst, src, scale=None):
                evict_ctr[0] += 1
                if evict_ctr[0] % 4 == 0:
                    if scale is None:
                        nc.vector.tensor_copy(dst, src)
                    else:
                        nc.vector.tensor_scalar_mul(dst, src, scale)
                else:
                    if scale is None:
                        nc.scalar.copy(dst, src)
                    else:
                        nc.scalar.mul(dst, src, scale)

            # ---------------- main loop over 128-sample tiles ----------------
            # Software-pipelined: phase A(mt) = DMAs + all projection GEMMs +
            # the DVE/ACT score->dmat chain; phase B(mt) = combine + WO + out.
            # Emitting B(mt-1) after A(mt) keeps the in-order PE queue busy on
            # tile mt's GEMMs while tile mt-1's score chain completes.

            def phase_a(mt):
                st = {}
                xt = xt_p.tile([128, 16, 128], FP8, tag="xt", name="xt")
                nc.sync.dma_start(xt[:], xtp_d[mt])
                xtd = xtd_p.tile([128, 8, 128], FP8, tag="xtd", name="xtd")
                nc.sync.dma_start(xtd[:], xtd_d[mt])
                xres = xres_p.tile([128, DIM], BF16, tag="xres", name="xres")
                nc.sync.dma_start(xres[:], xres_d[mt * 128:mt * 128 + 128, :])
                st["xres"] = xres

                # K-diffs: Kdiff_s = (T0 - T1) @ WKs, fp8 DoubleRow.
                # k1/k2 interleaved so both stream from one xtd weight load.
                kpsw = {k: ps_big.tile([128, SD], F32, tag="pbig", name="pbig")
                        for k in ("k1", "k2")}
                for fi in range(0, 8, 2):
                    for kname in ("k1", "k2"):
                        for n in range(2):
                            nsl = slice(n * 512, (n + 1) * 512)
                            nc.tensor.matmul(
                                kpsw[kname][:, nsl], xtd[:, fi:fi + 2, :],
                                w_sb[kname][:, fi:fi + 2, nsl],
                                start=(fi == 0), stop=(fi == 6),
                                perf_mode=DR)
                kdiff = {}
                for kname in ("k1", "k2"):
                    kd = kd_p.tile([128, SD], BF16, tag=f"kd_{kname}",
                                   name=f"kd_{kname}")
                    kdiff[kname] = kd
                    evict(kd[:], kpsw[kname][:])

                # Projections: q1/q2/v and the lam-MLP hidden GEMM interleaved
                # per token so all seven MM streams share each stationary xt
                # load.  lam accumulates across both tokens (full x row).
                ps_lam = ps_lam_p.tile([128, LH], F32, tag="plam", name="plam")
                proj = {}
                for tok in range(2):
                    psw = {}
                    for name in ("q1", "q2", "v"):
                        psw[name] = ps_big.tile([128, SD], F32, tag="pbig",
                                                name="pbig")
                    for fi in range(0, 8, 2):
                        lhsT = xt[:, tok * 8 + fi:tok * 8 + fi + 2, :]
                        for name in ("q1", "q2", "v"):
                            for n in range(2):
                                nsl = slice(n * 512, (n + 1) * 512)
                                nc.tensor.matmul(
                                    psw[name][:, nsl], lhsT,
                                    w_sb[name][:, fi:fi + 2, nsl],
                                    start=(fi == 0), stop=(fi == 6),
                                    perf_mode=DR)
                        nc.tensor.matmul(
                            ps_lam[:], lhsT,
                            l1_sb[:, tok * 8 + fi:tok * 8 + fi + 2, :],
                            start=(tok == 0 and fi == 0),
                            stop=(tok == 1 and fi == 6),
                            perf_mode=DR)
                    for name in ("q1", "q2", "v"):
                        pool = v_p if name == "v" else ptmp_p
                        sb = pool.tile([128, SD], BF16, tag=f"{name}_{tok}",
                                       name=f"{name}_{tok}")
                        proj[(name, tok)] = sb
                        evict(sb[:], psw[name][:],
                              scale=(1.0 / SV_EVICT) if name == "v" else None)
                st["proj"] = proj

                # lambda = sigmoid(relu(H)/64 . L2)
                h_sb = h_p.tile([128, LH], F32, tag="h", name="h")
                nc.scalar.activation(h_sb[:], ps_lam[:], ActFn.Relu,
                                     scale=1.0 / SW)
                nc.vector.tensor_tensor(h_sb[:], h_sb[:], l2_rep[:], AluOp.mult)
                logit = small_p.tile([128, 1], F32, tag="logit", name="logit")
                nc.vector.tensor_reduce(logit[:], h_sb[:],
                                        axis=mybir.AxisListType.X, op=AluOp.add)
                lam = small_p.tile([128, 1], F32, tag="lam", name="lam")
                nc.scalar.activation(lam[:], logit[:], ActFn.Sigmoid)
                u = small_p.tile([128, 1], F32, tag="u", name="u")
                nc.vector.tensor_scalar(u[:], lam[:], -1.0, 1.0,
                                        AluOp.mult, AluOp.add)

                # scores: r = <Q_q, Kdiff> per head; a = sigmoid(SCALE*r/4096)
                a = {}
                for si, sname in enumerate(("1", "2")):
                    for q in range(2):
                        p = ptmp_p.tile([128, SD], BF16, tag="p", name="p")
                        nc.vector.tensor_tensor(
                            p[:], proj[(f"q{sname}", q)][:],
                            kdiff[f"k{sname}"][:], AluOp.mult)
                        r = small_p.tile([128, H], F32, tag=f"r{si}{q}",
                                         name=f"r{si}{q}")
                        nc.vector.tensor_reduce(
                            r[:], p.rearrange("b (h d) -> b h d", h=H),
                            axis=mybir.AxisListType.X, op=AluOp.add)
                        aa = small_p.tile([128, H], F32, tag=f"a{si}{q}",
                                          name=f"a{si}{q}")
                        nc.scalar.activation(aa[:], r[:], ActFn.Sigmoid,
                                             scale=float(SCALE / (SW * SW)))
                        a[(si, q)] = aa

                # diff-attn weights -> diag matrices (bf16, on DVE).
                # Build order matches the combine's (ft, kv, q) consumption.
                wq = {}
                for q in range(2):
                    t = small_p.tile([128, H], F32, tag=f"t{q}", name=f"t{q}")
                    nc.vector.tensor_scalar_mul(t[:], a[(1, q)][:], lam[:])
                    w0q = small_p.tile([128, H], F32, tag=f"w0{q}",
                                       name=f"w0{q}")
                    nc.vector.tensor_tensor(w0q[:], a[(0, q)][:], t[:],
                                            AluOp.subtract)
                    nc.vector.tensor_scalar_max(w0q[:], w0q[:], 0.0)
                    w1q = small_p.tile([128, H], F32, tag=f"w1{q}",
                                       name=f"w1{q}")
                    nc.vector.tensor_tensor(w1q[:], t[:], a[(0, q)][:],
                                            AluOp.subtract)
                    nc.vector.tensor_scalar(w1q[:], w1q[:], u[:], 0.0,
                                            AluOp.add, AluOp.max)
                    wq[(q, 0)] = w0q
                    wq[(q, 1)] = w1q
                # dd[(h, kv)] = [D_q0 | D_q1]: one N=256 rhs per combine MM so
                # a single stationary V-slice load serves both q outputs.
                dmats = {}
                for h in range(H):
                    for kv in range(2):
                        dd = d_p.tile([128, 256], BF16, tag="dmat",
                                      name="dmat")
                        for q in range(2):
                            nc.vector.tensor_scalar_mul(
                                dd[:, q * 128:(q + 1) * 128], id_bf16[:],
                                wq[(q, kv)][:, h:h + 1])
                        dmats[(h, kv)] = dd
                st["dmats"] = dmats
                return st

            def phase_c(mt, st):
                # A_q^T via diag matmuls (bf16):
                #   A_q^T[ft] = V_0[:,ft].T @ D0[q,h] + V_1[:,ft].T @ D1[q,h]
                # PSUM holds both q outputs interleaved [ft, q, 128] so each
                # (ft, kv) is ONE N=256 MM with rhs [D_q0 | D_q1] — one
                # stationary V-slice load per MM, 16 MMs total.
                proj, dmats = st["proj"], st["dmats"]
                at = {}
                for q in range(2):
                    at[q] = at_p.tile([128, 8, 128], FP8, tag=f"at{q}",
                                      name=f"at{q}")
                psqA = ps_big.tile([128, SD], F32, tag="pbig", name="pbig")
                psqB = ps_big.tile([128, SD], F32, tag="pbig", name="pbig")
                for ft in range(8):
                    h = ft // 2
                    fsl = slice(ft * 128, (ft + 1) * 128)
                    psX = psqA if ft < 4 else psqB
                    col = (ft % 4) * 256
                    for kv in range(2):
                        nc.tensor.matmul(
                            psX[:, col:col + 256], proj[("v", kv)][:, fsl],
                            dmats[(h, kv)][:],
                            start=(kv == 0), stop=(kv == 1),
                            skip_group_check=(ft % 2 != 0))
                for q in range(2):
                    for half, psX in ((0, psqA), (1, psqB)):
                        src = psX[:].rearrange("b (f qq c) -> b f qq c",
                                               qq=2, c=128)[:, :, q, :]
                        nc.scalar.copy(at[q][:, half * 4:(half + 1) * 4, :],
                                       src)
                st["at"] = at

            def phase_w(mt, st):
                # out_q*512 = x_q*512 + A_q*8 @ WO*64 ; evict with scale 1/512
                # (residual x*512 added into PSUM by DVE; PE is the hot engine)
                at, xres = st["at"], st["xres"]
                r0 = mt * 128
                for q in range(2):
                    o_sb = o_p.tile([128, SD], F32, tag=f"o{q}", name=f"o{q}")
                    pso = ps_big.tile([128, SD], F32, tag="pbig", name="pbig")
                    for fi in range(0, 8, 2):
                        for n in range(2):
                            nsl = slice(n * 512, (n + 1) * 512)
                            nc.tensor.matmul(pso[:, nsl],
                                             at[q][:, fi:fi + 2, :],
                                             w_sb["o"][:, fi:fi + 2, nsl],
                                             start=(fi == 0), stop=(fi == 6),
                                             perf_mode=DR)
                    nc.vector.tensor_tensor(
                        pso[:], pso[:], xres[:, q * SD:(q + 1) * SD],
                        AluOp.add)
                    osl = slice(q * SD, (q + 1) * SD)
                    nc.scalar.mul(o_sb[:], pso[:], 1.0 / SO)
                    nc.sync.dma_start(out_d[r0:r0 + 128, osl], o_sb[:])

            # Two-stage software pipeline: emitting combine+WO of tile mt-1
            # after the projections of tile mt keeps the in-order PE queue
            # busy while tile mt-1's score chain completes on DVE/ACT.
            # (A deeper 3-stage split measured slightly worse on HW.)
            n_iters = n_mtiles * repeats
            prev = None
            for mt_rep in range(n_iters):
                mt = mt_rep % n_mtiles
                st = phase_a(mt)
                if prev is not None:
                    phase_c(prev[0], prev[1])
                    phase_w(prev[0], prev[1])
                prev = (mt, st)
            phase_c(prev[0], prev[1])
            phase_w(prev[0], prev[1])

    split_excess_waits(nc)
    return nc


_NC_CACHE = {}


def _get_nc(n_samples):
    if n_samples not in _NC_CACHE:
        _NC_CACHE[n_samples] = build_kernel(n_samples)
    return _NC_CACHE[n_samples]


def host_prep(inputs, n_samples=B_CORE):
    """Host-side shard + dtype/layout prep. Returns in_maps for 8 cores."""
    x = np.ascontiguousarray(np.asarray(inputs["x"], dtype=np.float32))
    assert x.shape[0] == N_CORES * n_samples and x.shape[1] == DIM
    f8 = ml_dtypes.float8_e4m3
    bf = ml_dtypes.bfloat16
    ws = {}
    for k in ("WQ1_w", "WK1_w", "WQ2_w", "WK2_w", "WV_w", "WO_w", "L1_w"):
        ws[k] = np.ascontiguousarray(
            (np.asarray(inputs[k], dtype=np.float32) * SW).astype(f8))
    l2rep = np.ascontiguousarray(
        np.broadcast_to(np.asarray(inputs["L2_w"], dtype=np.float32)
                        .reshape(1, LH), (128, LH)))
    n_mtiles = n_samples // 128
    # fp8 transposed x: [core, mt, b, ft, p] -> [core, mt, p, ft, b]
    x8 = x.astype(f8)
    xb = x8.reshape(N_CORES, n_mtiles, 128, 16, 128)
    xtp_all = np.ascontiguousarray(xb.transpose(0, 1, 4, 3, 2))
    # fp8 transposed token diff: diff in f32, then quantize
    xd = (x[:, :SD] - x[:, SD:]).astype(f8)
    xdb = xd.reshape(N_CORES, n_mtiles, 128, 8, 128)
    xtd_all = np.ascontiguousarray(xdb.transpose(0, 1, 4, 3, 2))
    # bf16 x*512 natural
    xres = (x * SO).astype(bf).reshape(N_CORES, n_samples, DIM)
    in_maps = []
    for c in range(N_CORES):
        m = {"xtp": xtp_all[c], "xtd": xtd_all[c],
             "xres": np.ascontiguousarray(xres[c]), "L2r": l2rep}
        m.update(ws)
        in_maps.append(m)
    return in_maps


def kernel(**inputs):
    from concourse.bass_utils import run_bass_kernel_spmd

    nc = _get_nc(B_CORE)
    in_maps = host_prep(inputs, B_CORE)
    res = run_bass_kernel_spmd(nc, in_maps, core_ids=list(range(N_CORES)))
    return np.concatenate([res.results[c]["out"] for c in range(N_CORES)], axis=0)
